# revision 1
# baseline (speedup 1.0000x reference)
"""Multi-level DWT (DB4) decomposition on 8 Trainium2 NeuronCores.

Strategy
--------
The reference applies, per level, a banded analysis matrix to the leading
L columns and deinterleaves even/odd outputs into [approx | detail].
Algebraically each level is a 4-tap stride-2 convolution along the column
axis:
    approx[t] = c0*x[2t] + c1*x[2t+1] + c2*x[2t+2] + c3*x[2t+3]
    detail[t] = c3*x[2t] - c2*x[2t+1] + c1*x[2t+2] - c0*x[2t+3]
with wraparound at level 0 (x[L], x[L+1] := x[0], x[1]) and zero-truncation
at deeper levels.  Rows are independent, so the batch dim shards across the
8 cores with zero communication (512 rows/core).

On-core, each tap is one accumulating TensorE matmul with a scaled identity
as the stationary operand (contraction = 128 batch rows) and a slice of x
as the moving operand, in float32r (full-rate, ~2^-12 rounding).  To keep
every matmul's moving operand CONTIGUOUS (stride-2 reads halve PE stream
rate), x is kept phase-split at every level: xe[t]=x[2t], xo[t]=x[2t+1].
Then approx = c0*xe + c1*xo + c2*xe[+1] + c3*xo[+1] — all contiguous
slices.  The phase split of the next level's input is folded into the
PSUM->SBUF approx copies (strided PSUM reads are free at the copies' 1x
rate); level 0 is split on the host.  Deep-level truncation needs no zero
padding: the s=2,3 tap matmuls of a level's last chunk are simply one
position shorter, leaving the correct 2-tap partial sum in PSUM.  Detail
chunks are copied to staging buffers that DMA straight out; levels with
L<=256 batch all 4 row-tiles into a single matmul via a 3-dim AP.
"""
import sys

if "/opt/trn_rl_repo" not in sys.path:
    sys.path.insert(0, "/opt/trn_rl_repo")

import numpy as np

import concourse.bacc as bacc
import concourse.mybir as mybir
from concourse import tile
from concourse.bass_utils import run_bass_kernel_spmd

DB4 = [0.4829629131445341, 0.8365163037378079, 0.2241438680420134,
       -0.1294095225512604]

B, N = 4096, 4096
NCORES = 8
RPC = B // NCORES        # rows per core = 512
P = 128                  # partitions
NRT = RPC // P           # row-tiles per core = 4
NLEV = 11                # int(log2(N)) - 1
SA = N + 2               # ping buffer region: [xe (N/2+1) | xo (N/2+1)]
SB = N // 2 + 2          # pong buffer region

F32 = mybir.dt.float32
F32R = mybir.dt.float32r

_nc_cache = {}


def _idents(taps_even, taps_odd):
    """[128, 8*128] fp32: 8 scaled identity matrices (4 even, 4 odd taps)."""
    w = np.zeros((P, 8 * P), dtype=np.float32)
    d = np.arange(P)
    for s in range(4):
        w[d, s * P + d] = taps_even[s]
        w[d, (4 + s) * P + d] = taps_odd[s]
    return w


def build_program(loop_iters=None, variant="full"):
    """Build + compile the per-core Bass program (identical on all cores).

    loop_iters: if given, wrap the whole body in tc.For_i for wall-clock
    timing amplification (used by test.py, not by the grading path).
    variant: "full" | "mm" (matmuls only, timing diagnostics).
    """
    key = (loop_iters, variant)
    if key in _nc_cache:
        return _nc_cache[key]
    mm_only = variant == "mm"

    nc = bacc.Bacc("TRN2", target_bir_lowering=False, debug=False)
    x_d = nc.dram_tensor("x", [RPC, SA], F32R, kind="ExternalInput").ap()
    w_d = nc.dram_tensor("w", [P, 8 * P], F32R, kind="ExternalInput").ap()
    y_d = nc.dram_tensor("y", [RPC, N], F32, kind="ExternalOutput").ap()

    with tile.TileContext(nc) as tc:
        with tc.tile_pool(name="sb", bufs=1) as sb, \
             tc.tile_pool(name="ps", bufs=8, space="PSUM") as ps:
            a_t = sb.tile([P, NRT * SA], F32R, name="a_t")     # levels 0,2,4..
            b_t = sb.tile([P, NRT * SB], F32R, name="b_t")     # levels 1,3,5..
            d0_t = sb.tile([P, NRT * (N // 2)], F32, name="d0_t")   # lvl0 detail
            d1_t = sb.tile([P, NRT * (N // 4)], F32, name="d1_t")   # lvl1 detail
            t_t = sb.tile([P, NRT * (N // 4)], F32, name="t_t")     # cols [0,1024)
            w_t = sb.tile([P, 8 * P], F32R, name="w_t")
            z_t = sb.tile([P, 2], F32, name="z_t")

            def body(_iv=None):
                nc.vector.memset(z_t[:], 0.0)
                nc.sync.dma_start(w_t[:], w_d)
                half = SA // 2
                for r in range(NRT):
                    if r == 0:
                        # HWDGE drains FIFO per issuing engine: lead with the
                        # small xe/xo pieces the first PE chunks read, so
                        # compute starts after ~0.5 MB instead of ~3 MB
                        pieces = [(0, 513), (half, half + 513),
                                  (513, 1025), (half + 513, half + 1025),
                                  (1025, half), (half + 1025, SA)]
                    else:
                        pieces = [(0, half), (half, SA)]
                    for lo, hi in pieces:
                        nc.sync.dma_start(
                            a_t[:, r * SA + lo:r * SA + hi],
                            x_d[r * P:(r + 1) * P, lo:hi])

                # warm the PE clock (HAM un-throttles after ~3.4 us of
                # activity) with dummy matmuls on the weights tile while
                # the input DMA is still in flight
                pw = ps.tile([P, 512], F32, name="pch", tag="ps")
                for _ in range(6):
                    nc.tensor.matmul(pw[:], w_t[:, 0:P], w_t[:, 0:512],
                                     start=True, stop=True)

                for lev in range(NLEV):
                    L = N >> lev                  # active length
                    Fh = L // 2                   # outputs per parity per row
                    src_t, s_str = (a_t, SA) if (lev % 2 == 0 or mm_only) \
                        else (b_t, SB)
                    dst_t, d_str = (b_t, SB) if lev % 2 == 0 else (a_t, SA)
                    if lev == 0:
                        det_t, det_str, det_off = d0_t, N // 2, 0
                    elif lev == 1:
                        det_t, det_str, det_off = d1_t, N // 4, 0
                    else:
                        det_t, det_str, det_off = t_t, N // 4, Fh
                    last = lev == NLEV - 1
                    Fn = Fh // 2                  # next level's per-parity len

                    if Fh == 256:
                        # pair row-tiles: 16 matmuls at fd=512 beat 32 at 256
                        sv = src_t[:].rearrange("p (r c) -> p r c", r=NRT)
                        dv = dst_t[:].rearrange("p (r c) -> p r c", r=NRT)
                        ev = det_t[:].rearrange("p (r c) -> p r c", r=NRT)
                        hs, hn = Fh + 1, Fn + 1
                        for r0 in (0, 2):
                            pe = ps.tile([P, 2 * Fh], F32, name="pch", tag="ps")
                            po = ps.tile([P, 2 * Fh], F32, name="pch", tag="ps")
                            for pt, wo in ((pe, 0), (po, 4)):
                                for s in range(4):
                                    off = (0 if s % 2 == 0 else hs) + s // 2
                                    rhs = sv[:, r0:r0 + 2, off:off + Fh]
                                    nc.tensor.matmul(
                                        pt[:],
                                        w_t[:, (wo + s) * P:(wo + s + 1) * P],
                                        rhs, start=(s == 0), stop=(s == 3))
                            if mm_only:
                                continue
                            pev = pe[:].rearrange("p (r c) -> p r c", r=2)
                            pov = po[:].rearrange("p (r c) -> p r c", r=2)
                            nc.scalar.copy(dv[:, r0:r0 + 2, 0:Fn],
                                           pev[:, :, 0:Fh:2])
                            nc.vector.tensor_copy(dv[:, r0:r0 + 2, hn:hn + Fn],
                                                  pev[:, :, 1:Fh:2])
                            nc.scalar.copy(
                                dv[:, r0:r0 + 2, Fn:Fn + 1],
                                z_t[:, 0:1].unsqueeze(1).to_broadcast([P, 2, 1]))
                            nc.scalar.copy(
                                dv[:, r0:r0 + 2, hn + Fn:hn + Fn + 1],
                                z_t[:, 0:1].unsqueeze(1).to_broadcast([P, 2, 1]))
                            nc.vector.tensor_copy(
                                ev[:, r0:r0 + 2, det_off:det_off + Fh], pov)
                    elif Fh >= 256:
                        nch = max(1, Fh // 512)
                        fd = min(Fh, 512)
                        for r in range(NRT):
                            ae = r * s_str
                            ao = r * s_str + Fh + 1
                            dae = r * d_str
                            dao = r * d_str + Fn + 1
                            for c in range(nch):
                                t0 = fd * c
                                # the s=2,3 taps of the last chunk read one
                                # cell past the data: the zero pad written by
                                # the previous level (host wrap cell at lev 0)
                                for par, wo in ((0, 0), (1, 4)):
                                    pt = ps.tile([P, fd], F32, name="pch",
                                                 tag="ps")
                                    if par == 0:
                                        pe = pt
                                    else:
                                        po = pt
                                    for s in range(4):
                                        off = (ae if s % 2 == 0 else ao) \
                                            + t0 + s // 2
                                        rhs = src_t[:, off:off + fd]
                                        nc.tensor.matmul(
                                            pt[:],
                                            w_t[:, (wo + s) * P:
                                                (wo + s + 1) * P],
                                            rhs, start=(s == 0), stop=(s == 3))
                                if mm_only:
                                    continue
                                # approx, phase-split for the next level
                                h = fd // 2
                                nc.scalar.copy(
                                    dst_t[:, dae + t0 // 2:dae + t0 // 2 + h],
                                    pe[:, 0:fd:2])
                                nc.vector.tensor_copy(
                                    dst_t[:, dao + t0 // 2:dao + t0 // 2 + h],
                                    pe[:, 1:fd:2])
                                eo = r * det_str + det_off + t0
                                if c % 2 == 0:
                                    nc.vector.tensor_copy(
                                        det_t[:, eo:eo + fd], po[:])
                                else:
                                    nc.scalar.copy(det_t[:, eo:eo + fd], po[:])
                            if not last and not mm_only:
                                # zero truncation pads for the next level
                                nc.scalar.copy(dst_t[:, dae + Fn:dae + Fn + 1],
                                               z_t[:, 0:1])
                                nc.scalar.copy(dst_t[:, dao + Fn:dao + Fn + 1],
                                               z_t[:, 0:1])
                    else:
                        # batch all row-tiles into one matmul: free = (NRT, Fh)
                        # levels >= 6 keep their input interleaved (natural):
                        # one contiguous approx copy; stride-2 reads are free
                        # at these sizes (fp32r is 4 cyc/row below fd=256)
                        in_nat = lev >= 6
                        sv = src_t[:].rearrange("p (r c) -> p r c", r=NRT)
                        hs = Fh + 1
                        pe = ps.tile([P, NRT * Fh], F32, name="pch", tag="ps")
                        po = ps.tile([P, NRT * Fh], F32, name="pch", tag="ps")
                        for pt, wo in ((pe, 0), (po, 4)):
                            for s in range(4):
                                if in_nat:
                                    rhs = sv[:, :, s:s + 2 * Fh - 1:2]
                                else:
                                    off = (0 if s % 2 == 0 else hs) + s // 2
                                    rhs = sv[:, :, off:off + Fh]
                                nc.tensor.matmul(
                                    pt[:],
                                    w_t[:, (wo + s) * P:(wo + s + 1) * P],
                                    rhs, start=(s == 0), stop=(s == 3))
                        if mm_only:
                            continue
                        pev = pe[:].rearrange("p (r c) -> p r c", r=NRT)
                        pov = po[:].rearrange("p (r c) -> p r c", r=NRT)
                        dv = dst_t[:].rearrange("p (r c) -> p r c", r=NRT)
                        ev = det_t[:].rearrange("p (r c) -> p r c", r=NRT)
                        if last:
                            # final approx (2 cols) in natural order
                            nc.scalar.copy(ev[:, :, 0:Fh], pev)
                        elif lev >= 5:
                            # next level reads natural: single contiguous copy
                            nc.scalar.copy(dv[:, :, 0:Fh], pev)
                            nc.scalar.copy(
                                dv[:, :, Fh:Fh + 2],
                                z_t[:].unsqueeze(1).to_broadcast([P, NRT, 2]))
                        else:
                            hn = Fn + 1
                            nc.scalar.copy(dv[:, :, 0:Fn], pev[:, :, 0:Fh:2])
                            nc.vector.tensor_copy(dv[:, :, hn:hn + Fn],
                                                  pev[:, :, 1:Fh:2])
                            nc.scalar.copy(
                                dv[:, :, Fn:Fn + 1],
                                z_t[:, 0:1].to_broadcast([P, NRT, 1]))
                            nc.scalar.copy(
                                dv[:, :, hn + Fn:hn + Fn + 1],
                                z_t[:, 0:1].to_broadcast([P, NRT, 1]))
                        nc.vector.tensor_copy(ev[:, :, det_off:det_off + Fh],
                                              pov)

                    # stream details out as soon as a level completes
                    if mm_only:
                        continue
                    if lev == 0:
                        nc.sync.dma_start(
                            y_d[:, N // 2:N].rearrange("(r p) c -> p r c", p=P),
                            d0_t[:].rearrange("p (r c) -> p r c", r=NRT))
                    elif lev == 1:
                        nc.sync.dma_start(
                            y_d[:, N // 4:N // 2].rearrange(
                                "(r p) c -> p r c", p=P),
                            d1_t[:].rearrange("p (r c) -> p r c", r=NRT))
                    elif Fh >= 64:
                        # per-level tail detail: final y cols [Fh, 2*Fh)
                        tv = t_t[:].rearrange("p (r c) -> p r c", r=NRT)
                        nc.sync.dma_start(
                            y_d[:, Fh:2 * Fh].rearrange(
                                "(r p) c -> p r c", p=P),
                            tv[:, :, Fh:2 * Fh])
                if not mm_only:
                    # remnant: levels with Fh < 64 plus the final approx
                    tv = t_t[:].rearrange("p (r c) -> p r c", r=NRT)
                    nc.sync.dma_start(
                        y_d[:, 0:64].rearrange("(r p) c -> p r c", p=P),
                        tv[:, :, 0:64])

            if loop_iters is None:
                body()
            else:
                with tc.For_i(0, loop_iters, 1,
                              hint_engines=(mybir.EngineType.PE,)) as iv:
                    body(iv)

    nc.compile()
    _nc_cache[key] = nc
    return nc


def _taps(W=None):
    if W is None:
        c = list(DB4)
    else:
        W = np.asarray(W)
        c = [float(W[i, 0]) for i in range(4)]
    return c, [c[3], -c[2], c[1], -c[0]]


def _phase_split(x):
    """[RPC, N] -> [RPC, SA]: [xe (N/2+1) | xo (N/2+1)] with wrap pads."""
    out = np.empty((x.shape[0], SA), dtype=np.float32)
    h = N // 2 + 1
    out[:, 0:h - 1] = x[:, 0::2]
    out[:, h - 1] = x[:, 0]
    out[:, h:2 * h - 1] = x[:, 1::2]
    out[:, 2 * h - 1] = x[:, 1]
    return out


def kernel(input, W=None, **_unused):
    x = np.ascontiguousarray(np.asarray(input), dtype=np.float32)
    assert x.shape == (B, N), x.shape
    te, to = _taps(W)
    w_np = _idents(te, to)
    in_maps = [{"x": _phase_split(x[c * RPC:(c + 1) * RPC]), "w": w_np}
               for c in range(NCORES)]
    nc = build_program()
    res = run_bass_kernel_spmd(nc, in_maps, core_ids=list(range(NCORES)))
    out = np.concatenate([res.results[c]["y"] for c in range(NCORES)], axis=0)
    return np.ascontiguousarray(out, dtype=np.float32)



# revision 4
# speedup vs baseline: 1.9152x; 1.9152x over previous
"""Multi-level DWT (DB4) decomposition on 8 Trainium2 NeuronCores.

Strategy (v2: phase-packed 64-row groups, fp16)
-----------------------------------------------
Each level is a 4-tap stride-2 conv along the column axis:
    approx[t] = c0*xe[t] + c1*xo[t] + c2*xe[t+1] + c3*xo[t+1]
    detail[t] = c3*xe[t] - c2*xo[t] + c1*xe[t+1] - c0*xo[t+1]
with xe[t]=x[2t], xo[t]=x[2t+1]; wraparound pads at level 0, zero pads
deeper.  Rows shard across the 8 cores (512 rows/core), zero comms.

On-core layout: rows are processed in 8 groups of 64.  A group's xe
lives on partitions 0-63 and its xo on partitions 64-127 of one SBUF
region, so a SINGLE matmul pair computes approx AND detail for 64 rows:
    mm1: stationary W1 (c0/c1 into approx rows, c3/-c2 into detail rows)
         x moving tile[:, t : t+fd]
    mm2: stationary W2 (c2/c3, c1/-c0) x tile[:, t+1 : t+1+fd], PSUM-acc.
That streams 2 moving columns per output column -- half the tap-matmul
scheme's 4 -- so PE time is ~8*sum(L) cycles ~= 27 us/core.  Everything
on-chip is fp16 (1 cyc/row at any free size, ample precision: taps and
N(0,1) data, fp32 PSUM accumulate), and the host packs/casts IO to fp16
so DMA traffic halves to ~8.4 MB/core.

PSUM->SBUF drains are the bottleneck-to-balance: approx deinterleaves
(strided PSUM reads, partition-crossing copy for the odd phase) feed the
next level's tile, details cast into fp16 staging for DMA-out.  GPSIMD
has no PSUM port, so these split between Activation (xe + ~61% of
detail) and DVE (xo + rest) to run just under PE.  Weight reloads
amortize by issuing all W1 matmuls of a 4-bank PSUM supertile before
the W2 pass.  Detail staging packs group pairs (even g -> partitions
0-63, odd g -> 64-127) so DMA-out rows land contiguously.
"""
import sys

if "/opt/trn_rl_repo" not in sys.path:
    sys.path.insert(0, "/opt/trn_rl_repo")

import numpy as np

import concourse.bacc as bacc
import concourse.mybir as mybir
from concourse import tile
from concourse.bass_utils import run_bass_kernel_spmd

DB4 = [0.4829629131445341, 0.8365163037378079, 0.2241438680420134,
       -0.1294095225512604]

B, N = 4096, 4096
NCORES = 8
RPC = B // NCORES        # rows per core = 512
P = 128                  # partitions
G = 8                    # row groups per core (64 rows each)
NLEV = 11                # int(log2(N)) - 1
W0 = N // 2 + 1          # level-0 per-group region width (xe|xo + pad)

F32 = mybir.dt.float32
F16 = mybir.dt.float16

_nc_cache = {}


def build_program(loop_iters=None, variant="full"):
    """Build + compile the per-core Bass program (identical on all cores).

    loop_iters: if given, wrap the body in tc.For_i for wall-clock timing
    amplification (used by test.py, not by the grading path).
    variant: "full" | "mm" (matmuls only, timing diagnostics).
    """
    key = (loop_iters, variant)
    if key in _nc_cache:
        return _nc_cache[key]
    mm_only = variant == "mm"

    nc = bacc.Bacc("TRN2", target_bir_lowering=False, debug=False)
    x_d = nc.dram_tensor("x", [P, G * W0], F16, kind="ExternalInput").ap()
    w_d = nc.dram_tensor("w", [P, 256], F16, kind="ExternalInput").ap()
    y_d = nc.dram_tensor("y", [RPC, N], F16, kind="ExternalOutput").ap()

    with tile.TileContext(nc) as tc:
        with tc.tile_pool(name="sb", bufs=1) as sb, \
             tc.tile_pool(name="ps", bufs=2, space="PSUM") as ps:
            a_t = sb.tile([P, G * W0], F16, name="a_t")        # lev 0,2,4..
            b_t = sb.tile([P, G * (N // 4 + 1)], F16, name="b_t")  # 1,3,5..
            d0_t = sb.tile([P, 4 * (N // 2)], F16, name="d0_t")    # lev0 det
            d1_t = sb.tile([P, 4 * (N // 4)], F16, name="d1_t")    # lev1 det
            t2_t = sb.tile([P, 4 * (N // 4)], F16, name="t2_t")    # cols<1024
            w_t = sb.tile([P, 256], F16, name="w_t")

            yv = y_d.rearrange("(gg q) c -> q gg c", q=P)      # [128,4,4096]

            def body(_iv=None):
                nc.sync.dma_start(w_t[:], w_d)
                # group-0 slab in 4 pieces so PE starts after ~130 KB
                pieces = [(0, 514), (514, 1026), (1026, 1538), (1538, W0)]
                pieces += [(g * W0, (g + 1) * W0) for g in range(1, G)]
                for lo, hi in pieces:
                    nc.sync.dma_start(a_t[:, lo:hi], x_d[:, lo:hi])

                for lev in range(NLEV):
                    Li = N >> lev
                    Fi = Li >> 1             # outputs per parity per row
                    Fn = Fi >> 1
                    Wi = Fi + 1              # src per-group region width
                    Wn = Fn + 1
                    src_t = a_t if (lev % 2 == 0 or mm_only) else b_t
                    dst_t = b_t if lev % 2 == 0 else a_t
                    last = lev == NLEV - 1
                    if lev == 0:
                        det_t, det_w, det_b = d0_t, N // 2, 0
                    elif lev == 1:
                        det_t, det_w, det_b = d1_t, N // 4, 0
                    else:
                        det_t, det_w, det_b = t2_t, N // 4, Fi
                    ng = min(G, max(1, 2048 // Fi))   # groups per supertile
                    gb = max(1, min(ng, 512 // Fi))   # groups per matmul
                    sv = src_t[:, 0:G * Wi].rearrange("p (g w) -> p g w", g=G)
                    if not last:
                        dv = dst_t[:, 0:G * Wn].rearrange(
                            "p (g w) -> p g w", g=G)
                        if not mm_only:
                            # zero truncation pads for the next level
                            nc.gpsimd.memset(dv[:, :, Fn:Fn + 1], 0.0)
                    ev = det_t[:, 0:4 * det_w].rearrange(
                        "p (gg c) -> p gg c", gg=4)
                    # Act takes xe + ~61% of detail, DVE takes xo + rest
                    asp = min(Fi, (int(0.61 * Fi) + 1) & ~1)

                    for ga in range(0, G, ng):
                        pt = ps.tile([P, ng * Fi], F32, name="pst", tag="ps")
                        # all W1 matmuls, then all W2 (amortize LD_WEIGHTS)
                        for sh, wsl in ((0, w_t[:, 0:128]),
                                        (1, w_t[:, 128:256])):
                            if Fi >= 512:
                                for gi in range(ng):
                                    for t0 in range(0, Fi, 512):
                                        o = gi * Fi + t0
                                        nc.tensor.matmul(
                                            pt[:, o:o + 512], wsl,
                                            sv[:, ga + gi:ga + gi + 1,
                                               sh + t0:sh + t0 + 512],
                                            start=(sh == 0), stop=(sh == 1))
                            else:
                                for gm in range(0, ng, gb):
                                    nc.tensor.matmul(
                                        pt[:, gm * Fi:(gm + gb) * Fi], wsl,
                                        sv[:, ga + gm:ga + gm + gb,
                                           sh:sh + Fi],
                                        start=(sh == 0), stop=(sh == 1))
                        if mm_only:
                            continue
                        pv = pt[:].rearrange("p (g f) -> p g f", g=ng)
                        if last:
                            # final approx (2 cols, natural order) -> cols 0:2
                            nc.scalar.copy(ev[0:64, :, 0:2],
                                           pv[0:64, 0::2, :])
                            nc.vector.tensor_copy(ev[64:128, :, 0:2],
                                                  pv[0:64, 1::2, :])
                        else:
                            # approx, phase-split for the next level
                            nc.scalar.copy(dv[0:64, ga:ga + ng, 0:Fn],
                                           pv[0:64, :, 0:Fi:2])
                            nc.vector.tensor_copy(
                                dv[64:128, ga:ga + ng, 0:Fn],
                                pv[0:64, :, 1:Fi:2])
                        # details -> staging (group pairs pack 128 partitions)
                        if ng == 1:
                            p2, gg = ga % 2, ga // 2
                            nc.scalar.copy(
                                ev[p2 * 64:p2 * 64 + 64, gg:gg + 1,
                                   det_b:det_b + asp],
                                pv[64:128, 0:1, 0:asp])
                            nc.vector.tensor_copy(
                                ev[p2 * 64:p2 * 64 + 64, gg:gg + 1,
                                   det_b + asp:det_b + Fi],
                                pv[64:128, 0:1, asp:Fi])
                        else:
                            for p2 in (0, 1):
                                gg = (ga + p2) // 2
                                n2 = ng // 2
                                dsl = ev[p2 * 64:p2 * 64 + 64, gg:gg + n2]
                                ssl = pv[64:128, p2::2]
                                nc.scalar.copy(
                                    dsl[:, :, det_b:det_b + asp],
                                    ssl[:, :, 0:asp])
                                if asp < Fi:
                                    nc.vector.tensor_copy(
                                        dsl[:, :, det_b + asp:det_b + Fi],
                                        ssl[:, :, asp:Fi])

                    if mm_only:
                        continue
                    # stream details out as soon as a level completes
                    if lev == 0:
                        e0 = d0_t[:].rearrange("p (gg c) -> p gg c", gg=4)
                        nc.sync.dma_start(yv[:, 0:2, N // 2:N], e0[:, 0:2])
                        nc.sync.dma_start(yv[:, 2:4, N // 2:N], e0[:, 2:4])
                    elif lev == 1:
                        e1 = d1_t[:].rearrange("p (gg c) -> p gg c", gg=4)
                        nc.sync.dma_start(yv[:, 0:2, N // 4:N // 2], e1[:, 0:2])
                        nc.sync.dma_start(yv[:, 2:4, N // 4:N // 2], e1[:, 2:4])
                    elif Fi >= 64:
                        nc.sync.dma_start(yv[:, :, Fi:2 * Fi],
                                          ev[:, :, Fi:2 * Fi])
                if not mm_only:
                    # remnant: levels with Fi < 64 plus the final approx
                    tv = t2_t[:].rearrange("p (gg c) -> p gg c", gg=4)
                    nc.sync.dma_start(yv[:, :, 0:64], tv[:, :, 0:64])

            if loop_iters is None:
                body()
            else:
                with tc.For_i(0, loop_iters, 1,
                              hint_engines=(mybir.EngineType.PE,)) as iv:
                    body(iv)

    nc.compile()
    _nc_cache[key] = nc
    return nc


def _taps(W=None):
    if W is None:
        return list(DB4)
    W = np.asarray(W)
    return [float(W[i, 0]) for i in range(4)]


def _wmats(c):
    """[128, 256] fp16: [W1 | W2] stationaries (see module docstring)."""
    w = np.zeros((P, 256), dtype=np.float32)
    r = np.arange(64)
    w[r, r] = c[0]
    w[64 + r, r] = c[1]
    w[r, 64 + r] = c[3]
    w[64 + r, 64 + r] = -c[2]
    w[r, 128 + r] = c[2]
    w[64 + r, 128 + r] = c[3]
    w[r, 192 + r] = c[1]
    w[64 + r, 192 + r] = -c[0]
    return w.astype(np.float16)


def _pack_input(x):
    """[RPC, N] fp32 -> [128, G*W0] fp16 phase-packed groups with wrap pads."""
    xr = x.reshape(G, 64, N)
    out = np.empty((P, G, W0), dtype=np.float16)
    out[0:64, :, 0:N // 2] = xr[:, :, 0::2].transpose(1, 0, 2)
    out[64:128, :, 0:N // 2] = xr[:, :, 1::2].transpose(1, 0, 2)
    out[0:64, :, N // 2] = xr[:, :, 0].T
    out[64:128, :, N // 2] = xr[:, :, 1].T
    return out.reshape(P, G * W0)


def make_in_maps(input, W=None):
    x = np.ascontiguousarray(np.asarray(input), dtype=np.float32)
    assert x.shape == (B, N), x.shape
    w_np = _wmats(_taps(W))
    return [{"x": _pack_input(x[c * RPC:(c + 1) * RPC]), "w": w_np}
            for c in range(NCORES)]


def kernel(input, W=None, **_unused):
    in_maps = make_in_maps(input, W)
    nc = build_program()
    res = run_bass_kernel_spmd(nc, in_maps, core_ids=list(range(NCORES)))
    out = np.concatenate([res.results[c]["y"].astype(np.float32)
                          for c in range(NCORES)], axis=0)
    return np.ascontiguousarray(out, dtype=np.float32)
